# revision 10
# baseline (speedup 1.0000x reference)
"""Grouped-experts SwiGLU MoE kernel for Trainium2 (8 NeuronCores).

Expert-parallel sharding: core e owns expert e's weights and its contiguous
token group (m_sizes gives T//E = 2048 tokens per expert). No collectives —
routing/scatter/gather happens on the host, each core runs an identical
single-core program on its own shard.

Per-core math: out = (silu(x_e @ w1_e) * (x_e @ w3_e)) @ w2_e
  x_e [2048, 2048], w1/w3 [2048, 1024], w2 [1024, 2048].

Device strategy (all bf16 inputs, f32 PSUM accumulation): all weights and
the full xT shard are SBUF-resident (8 + 4 + 8 MB); tokens are processed
in 4 groups of 512 so the first matmul only waits for ~2MB of DMA:

  per group g: phase 1 (up+gate): stationary = w1/w3 128x128 tiles,
      moving = xT[:, :, group] (D on partitions). PSUM accumulates over D;
      SwiGLU evac (ACT silu + DVE mul) writes zT [H, 512] as bf16.
      phase 2 (down): stationary = zT 128x128 tiles, moving = w2.
      PSUM accumulates over H; DVE/ACT alternate casting the result to
      bf16 SBUF and DMA stores out [M, D] rows (host upcasts to f32).

PSUM budget: ph1 uses tags p0/p1 (1 bank per u/g), ph2 tags p0..p3, pool
bufs=2 -> exactly 8 banks with cross-iteration double buffering.
A burst of dependency-free warmup matmuls covers the ~9us DMA spin-up so
the PE clock is un-throttled (HAM K=8/8) when real work arrives.
"""

import numpy as np
import ml_dtypes

E, T, D, H = 8, 16384, 2048, 1024
M = T // E            # tokens per expert
P = 128
DC = D // P           # 16 contraction chunks (phase 1)
HC = H // P           # 8 contraction chunks (phase 2)
NG = 4                # token groups
MG = M // NG          # 512 tokens per group
NMOV = 512            # moving free dim / PSUM bank width (f32)
G = 8                 # d-chunks per weight-stream DMA (256KB transfers)
NWARM = 16

_CACHE = {}
LAST_RESULTS = None   # for test harnesses that want the profile


def _build_program():
    import concourse.bacc as bacc
    import concourse.bass as bass
    import concourse.mybir as mybir
    import concourse.tile as tile

    f32 = mybir.dt.float32
    bf16 = mybir.dt.bfloat16
    SILU = mybir.ActivationFunctionType.Silu

    nc = bacc.Bacc("TRN2", target_bir_lowering=False, debug=False)

    xT = nc.dram_tensor("xT", [D, M], bf16, kind="ExternalInput")
    w1r = nc.dram_tensor("w1r", [HC, DC // G, P, G, P], bf16, kind="ExternalInput")
    w3r = nc.dram_tensor("w3r", [HC, DC // G, P, G, P], bf16, kind="ExternalInput")
    w2r = nc.dram_tensor("w2r", [HC, P, D], bf16, kind="ExternalInput")
    out = nc.dram_tensor("out", [M, D], bf16, kind="ExternalOutput")

    xT_t = xT.rearrange("(c p) m -> p c m", p=P)  # [P, DC, M]

    with tile.TileContext(nc) as tc:
        with (
            tc.tile_pool(name="xp", bufs=1) as xp,
            tc.tile_pool(name="wp", bufs=1) as wp,
            tc.tile_pool(name="zp", bufs=2) as zp,
            tc.tile_pool(name="op", bufs=2) as op,
            tc.tile_pool(name="sp", bufs=3) as sp,
            tc.tile_pool(name="ps", bufs=2, space=bass.MemorySpace.PSUM) as ps,
        ):
            # resident tensors: full x shard + all weights
            xt = xp.tile([P, DC, M], bf16, tag="xt")       # 8MB
            w1s = wp.tile([P, HC, DC, P], bf16, tag="w1")  # 4MB
            w3s = wp.tile([P, HC, DC, P], bf16, tag="w3")  # 4MB
            w2t = wp.tile([P, HC, D], bf16, tag="w2")      # 4MB

            # HAM warmup: dependency-free matmuls on a memset tile keep the
            # PE busy during the ~9us initial DMA wait so the tensor clock
            # is already un-throttled (K=8/8) when the first real matmul's
            # inputs land.
            wu = sp.tile([P, 256], bf16, tag="wu")
            nc.vector.memset(wu[:], 0)
            pw = ps.tile([P, NMOV], f32, tag="p0", name="warm")
            for _ in range(NWARM):
                nc.tensor.matmul(
                    pw[:, :256], wu[:, :P], wu[:],
                    start=True, stop=True, skip_group_check=True,
                )

            def load_w13(h, cg):
                csl = slice(cg * G, (cg + 1) * G)
                nc.sync.dma_start(w1s[:, h, csl, :], w1r[h, cg])
                nc.sync.dma_start(w3s[:, h, csl, :], w3r[h, cg])

            def load_x(gr, c):
                msl = slice(gr * MG, (gr + 1) * MG)
                nc.sync.dma_start(xt[:, c, msl], xT_t[:, c, msl])

            for gr in range(NG):
                zt = zp.tile([P, HC, MG], bf16, tag="zt")

                # ---- phase 1: u = x@w1, g = x@w3, z = silu(u)*g ----
                for h in range(HC):
                    if gr == 0:
                        # group 0 drives all input DMA, in consumption
                        # order. The w13 stream runs one h-iteration ahead
                        # of its consumer so an h-start never waits on its
                        # own weight tiles; group-1 x and w2 prefetch ride
                        # behind it.
                        if h == 0:
                            load_w13(0, 0)
                            for c in range(G):
                                load_x(0, c)
                            load_w13(0, 1)
                            for c in range(G, DC):
                                load_x(0, c)
                        if h < HC - 1:
                            for cg in range(DC // G):
                                load_w13(h + 1, cg)
                        if h >= 2:
                            for c in range(h - 2, DC, HC - 2):
                                load_x(1, c)
                        # w2 slices go behind the w13 stream (a 4MB load
                        # earlier would delay the weight tiles in the
                        # DMA queue and stall phase 1) but early enough to
                        # beat phase 2's h-consumption.
                        if h >= 5:
                            for hh in range((h - 5) * 3, min((h - 4) * 3, HC)):
                                nc.sync.dma_start(w2t[:, hh, :], w2r[hh])
                    elif gr < NG - 1:
                        # prefetch next group's x two h-iterations ahead
                        for c in range(2 * h, 2 * h + 2):
                            load_x(gr + 1, c)

                    pu = ps.tile([P, NMOV], f32, tag="p0", name="pu")
                    pg = ps.tile([P, NMOV], f32, tag="p1", name="pg")
                    msl = slice(gr * MG, (gr + 1) * MG)
                    for c in range(DC):
                        first, last = c == 0, c == DC - 1
                        nc.tensor.matmul(
                            pu[:], w1s[:, h, c, :], xt[:, c, msl],
                            start=first, stop=last,
                        )
                        nc.tensor.matmul(
                            pg[:], w3s[:, h, c, :], xt[:, c, msl],
                            start=first, stop=last,
                        )
                        # trickle zone (group 0, first two h): the x/w
                        # stream paces the PE, and the resulting sub-us
                        # idle slots re-throttle the HAM clock right as
                        # the stream turns PE-bound. Dependency-free
                        # filler matmuls keep the PE activity window busy;
                        # they cost nothing while DMA is behind.
                        if gr == 0 and (h == 0 or (h == 1 and c % 2 == 1)):
                            nc.tensor.matmul(
                                pw[:, :256], wu[:, :P], wu[:],
                                start=True, stop=True, skip_group_check=True,
                            )
                    st = sp.tile([P, NMOV], f32, tag="st")
                    nc.scalar.activation(st[:], pu[:], SILU)
                    nc.vector.tensor_mul(zt[:, h, :], st[:], pg[:])

                # ---- phase 2: out = z @ w2 ----
                for mi in range(MG // P):
                    po = [ps.tile([P, NMOV], f32, tag=f"p{dd}", name=f"po{dd}")
                          for dd in range(4)]
                    for h in range(HC):
                        zst = zt[:, h, mi * P:(mi + 1) * P]
                        for dd in range(D // NMOV):
                            nc.tensor.matmul(
                                po[dd][:], zst,
                                w2t[:, h, dd * NMOV:(dd + 1) * NMOV],
                                start=h == 0, stop=h == HC - 1,
                            )
                    osb = op.tile([P, D], bf16, tag="o")
                    r0 = gr * MG + mi * P
                    for dd in range(D // NMOV):
                        # alternate PSUM-evac casts between DVE and ACT to
                        # halve the final evac chain on the critical path
                        dst = osb[:, dd * NMOV:(dd + 1) * NMOV]
                        if dd % 2 == 0:
                            nc.vector.tensor_copy(dst, po[dd][:])
                        else:
                            nc.scalar.copy(dst, po[dd][:])
                        nc.sync.dma_start(
                            out[r0:r0 + P, dd * NMOV:(dd + 1) * NMOV],
                            dst,
                        )

    nc.compile()
    return nc


def _get_program():
    if "nc" not in _CACHE:
        _CACHE["nc"] = _build_program()
    return _CACHE["nc"]


def _prep_w13(w):
    # [D, H] -> [HC, DC//G, P, G, P]; element [h,cg,p,g,m] = w[(cg*G+g)*P+p, h*P+m]
    return np.ascontiguousarray(
        w.reshape(DC // G, G, P, HC, P).transpose(3, 0, 2, 1, 4)
    )


def _numpy_fallback(x, w1, w2, w3, m_sizes):
    offs = np.concatenate([[0], np.cumsum(np.asarray(m_sizes, dtype=np.int64))])
    out = np.zeros((x.shape[0], w2.shape[2]), dtype=np.float32)
    for e in range(w1.shape[0]):
        xe = x[offs[e]:offs[e + 1]]
        u = xe @ w1[e]
        g = xe @ w3[e]
        z = (u / (1.0 + np.exp(-u))) * g
        out[offs[e]:offs[e + 1]] = z @ w2[e]
    return out


def kernel(x, w1, w2, w3, m_sizes, _trace=False, _trace_kwargs=None):
    global LAST_RESULTS
    x = np.ascontiguousarray(x, dtype=np.float32)
    w1 = np.ascontiguousarray(w1, dtype=np.float32)
    w2 = np.ascontiguousarray(w2, dtype=np.float32)
    w3 = np.ascontiguousarray(w3, dtype=np.float32)
    m = np.asarray(m_sizes, dtype=np.int64)

    expected = (
        x.shape == (T, D)
        and w1.shape == (E, D, H)
        and w2.shape == (E, H, D)
        and w3.shape == (E, D, H)
        and m.shape == (E,)
        and np.all(m == M)
    )
    if not expected:
        return _numpy_fallback(x, w1, w2, w3, m_sizes)

    from concourse.bass_utils import run_bass_kernel_spmd

    bf = ml_dtypes.bfloat16
    nc = _get_program()
    in_maps = []
    for e in range(E):
        in_maps.append({
            "xT": np.ascontiguousarray(x[e * M:(e + 1) * M].T.astype(bf)),
            "w1r": _prep_w13(w1[e].astype(bf)),
            "w3r": _prep_w13(w3[e].astype(bf)),
            "w2r": np.ascontiguousarray(
                w2[e].astype(bf).reshape(HC, P, D)
            ),
        })

    res = run_bass_kernel_spmd(
        nc, in_maps, core_ids=list(range(E)),
        trace=_trace, **(_trace_kwargs or {}),
    )
    LAST_RESULTS = res
    return np.concatenate(
        [r["out"].astype(np.float32) for r in res.results], axis=0
    )


# revision 11
# speedup vs baseline: 1.0064x; 1.0064x over previous
"""Grouped-experts SwiGLU MoE kernel for Trainium2 (8 NeuronCores).

Expert-parallel sharding: core e owns expert e's weights and its contiguous
token group (m_sizes gives T//E = 2048 tokens per expert). No collectives —
routing/scatter/gather happens on the host, each core runs an identical
single-core program on its own shard.

Per-core math: out = (silu(x_e @ w1_e) * (x_e @ w3_e)) @ w2_e
  x_e [2048, 2048], w1/w3 [2048, 1024], w2 [1024, 2048].

Device strategy (all bf16 inputs, f32 PSUM accumulation): all weights and
the full xT shard are SBUF-resident (8 + 4 + 8 MB); tokens are processed
in 4 groups of 512 so the first matmul only waits for ~2MB of DMA:

  per group g: phase 1 (up+gate): stationary = w1/w3 128x128 tiles,
      moving = xT[:, :, group] (D on partitions). PSUM accumulates over D;
      SwiGLU evac (ACT silu + DVE mul) writes zT [H, 512] as bf16.
      phase 2 (down): stationary = zT 128x128 tiles, moving = w2.
      PSUM accumulates over H; DVE/ACT alternate casting the result to
      bf16 SBUF and DMA stores out [M, D] rows (host upcasts to f32).

PSUM budget: ph1 uses tags p0/p1 (1 bank per u/g), ph2 tags p0..p3, pool
bufs=2 -> exactly 8 banks with cross-iteration double buffering.
A burst of dependency-free warmup matmuls covers the ~9us DMA spin-up so
the PE clock is un-throttled (HAM K=8/8) when real work arrives.
"""

import numpy as np
import ml_dtypes

E, T, D, H = 8, 16384, 2048, 1024
M = T // E            # tokens per expert
P = 128
DC = D // P           # 16 contraction chunks (phase 1)
HC = H // P           # 8 contraction chunks (phase 2)
NG = 4                # token groups
MG = M // NG          # 512 tokens per group
NMOV = 512            # moving free dim / PSUM bank width (f32)
G = 8                 # d-chunks per weight-stream DMA (256KB transfers)
NWARM = 16

_CACHE = {}
LAST_RESULTS = None   # for test harnesses that want the profile


def _build_program():
    import concourse.bacc as bacc
    import concourse.bass as bass
    import concourse.mybir as mybir
    import concourse.tile as tile

    f32 = mybir.dt.float32
    bf16 = mybir.dt.bfloat16
    SILU = mybir.ActivationFunctionType.Silu

    nc = bacc.Bacc("TRN2", target_bir_lowering=False, debug=False)

    xT = nc.dram_tensor("xT", [D, M], bf16, kind="ExternalInput")
    w1r = nc.dram_tensor("w1r", [HC, DC // G, P, G, P], bf16, kind="ExternalInput")
    w3r = nc.dram_tensor("w3r", [HC, DC // G, P, G, P], bf16, kind="ExternalInput")
    w2r = nc.dram_tensor("w2r", [HC, P, D], bf16, kind="ExternalInput")
    out = nc.dram_tensor("out", [M, D], bf16, kind="ExternalOutput")

    xT_t = xT.rearrange("(c p) m -> p c m", p=P)  # [P, DC, M]

    with tile.TileContext(nc) as tc:
        with (
            tc.tile_pool(name="xp", bufs=1) as xp,
            tc.tile_pool(name="wp", bufs=1) as wp,
            tc.tile_pool(name="zp", bufs=2) as zp,
            tc.tile_pool(name="op", bufs=2) as op,
            tc.tile_pool(name="sp", bufs=3) as sp,
            tc.tile_pool(name="ps", bufs=2, space=bass.MemorySpace.PSUM) as ps,
        ):
            # resident tensors: full x shard + all weights
            xt = xp.tile([P, DC, M], bf16, tag="xt")       # 8MB
            w1s = wp.tile([P, HC, DC, P], bf16, tag="w1")  # 4MB
            w3s = wp.tile([P, HC, DC, P], bf16, tag="w3")  # 4MB
            w2t = wp.tile([P, HC, D], bf16, tag="w2")      # 4MB

            # HAM warmup: dependency-free matmuls on a memset tile keep the
            # PE busy during the ~9us initial DMA wait so the tensor clock
            # is already un-throttled (K=8/8) when the first real matmul's
            # inputs land.
            wu = sp.tile([P, 256], bf16, tag="wu")
            nc.vector.memset(wu[:], 0)
            pw = ps.tile([P, NMOV], f32, tag="p0", name="warm")
            for _ in range(NWARM):
                nc.tensor.matmul(
                    pw[:, :256], wu[:, :P], wu[:],
                    start=True, stop=True, skip_group_check=True,
                )

            def load_w13(h, cg):
                csl = slice(cg * G, (cg + 1) * G)
                nc.sync.dma_start(w1s[:, h, csl, :], w1r[h, cg])
                nc.sync.dma_start(w3s[:, h, csl, :], w3r[h, cg])

            def load_x(gr, c):
                msl = slice(gr * MG, (gr + 1) * MG)
                nc.sync.dma_start(xt[:, c, msl], xT_t[:, c, msl])

            for gr in range(NG):
                zt = zp.tile([P, HC, MG], bf16, tag="zt")

                # ---- phase 1: u = x@w1, g = x@w3, z = silu(u)*g ----
                for h in range(HC):
                    if gr == 0:
                        # group 0 drives all input DMA, in consumption
                        # order. The w13 stream runs one h-iteration ahead
                        # of its consumer so an h-start never waits on its
                        # own weight tiles; group-1 x and w2 prefetch ride
                        # behind it.
                        if h == 0:
                            load_w13(0, 0)
                            for c in range(G):
                                load_x(0, c)
                            load_w13(0, 1)
                            for c in range(G, DC):
                                load_x(0, c)
                        if h < HC - 1:
                            for cg in range(DC // G):
                                load_w13(h + 1, cg)
                        if h >= 2:
                            for c in range(h - 2, DC, HC - 2):
                                load_x(1, c)
                        # w2 slices go behind the w13 stream (a 4MB load
                        # earlier would delay the weight tiles in the
                        # DMA queue and stall phase 1) but early enough to
                        # beat phase 2's h-consumption.
                        if h >= 5:
                            for hh in range((h - 5) * 3, min((h - 4) * 3, HC)):
                                nc.sync.dma_start(w2t[:, hh, :], w2r[hh])
                    elif gr < NG - 1:
                        # prefetch next group's x two h-iterations ahead
                        for c in range(2 * h, 2 * h + 2):
                            load_x(gr + 1, c)

                    pu = ps.tile([P, NMOV], f32, tag="p0", name="pu")
                    pg = ps.tile([P, NMOV], f32, tag="p1", name="pg")
                    msl = slice(gr * MG, (gr + 1) * MG)
                    for c in range(DC):
                        first, last = c == 0, c == DC - 1
                        nc.tensor.matmul(
                            pu[:], w1s[:, h, c, :], xt[:, c, msl],
                            start=first, stop=last,
                        )
                        nc.tensor.matmul(
                            pg[:], w3s[:, h, c, :], xt[:, c, msl],
                            start=first, stop=last,
                        )
                        # trickle zone (group 0, first two h): the x/w
                        # stream paces the PE, and the resulting sub-us
                        # idle slots re-throttle the HAM clock right as
                        # the stream turns PE-bound. Dependency-free
                        # filler matmuls keep the PE activity window busy;
                        # they cost nothing while DMA is behind.
                        if gr == 0 and h == 0:
                            nc.tensor.matmul(
                                pw[:, :256], wu[:, :P], wu[:],
                                start=True, stop=True, skip_group_check=True,
                            )
                    st = sp.tile([P, NMOV], f32, tag="st")
                    nc.scalar.activation(st[:], pu[:], SILU)
                    nc.vector.tensor_mul(zt[:, h, :], st[:], pg[:])

                # ---- phase 2: out = z @ w2 ----
                for mi in range(MG // P):
                    po = [ps.tile([P, NMOV], f32, tag=f"p{dd}", name=f"po{dd}")
                          for dd in range(4)]
                    for h in range(HC):
                        zst = zt[:, h, mi * P:(mi + 1) * P]
                        for dd in range(D // NMOV):
                            nc.tensor.matmul(
                                po[dd][:], zst,
                                w2t[:, h, dd * NMOV:(dd + 1) * NMOV],
                                start=h == 0, stop=h == HC - 1,
                            )
                    osb = op.tile([P, D], bf16, tag="o")
                    r0 = gr * MG + mi * P
                    for dd in range(D // NMOV):
                        # alternate PSUM-evac casts between DVE and ACT to
                        # halve the final evac chain on the critical path
                        dst = osb[:, dd * NMOV:(dd + 1) * NMOV]
                        if dd % 2 == 0:
                            nc.vector.tensor_copy(dst, po[dd][:])
                        else:
                            nc.scalar.copy(dst, po[dd][:])
                        nc.sync.dma_start(
                            out[r0:r0 + P, dd * NMOV:(dd + 1) * NMOV],
                            dst,
                        )

    nc.compile()
    return nc


def _get_program():
    if "nc" not in _CACHE:
        _CACHE["nc"] = _build_program()
    return _CACHE["nc"]


def _prep_w13(w):
    # [D, H] -> [HC, DC//G, P, G, P]; element [h,cg,p,g,m] = w[(cg*G+g)*P+p, h*P+m]
    return np.ascontiguousarray(
        w.reshape(DC // G, G, P, HC, P).transpose(3, 0, 2, 1, 4)
    )


def _numpy_fallback(x, w1, w2, w3, m_sizes):
    offs = np.concatenate([[0], np.cumsum(np.asarray(m_sizes, dtype=np.int64))])
    out = np.zeros((x.shape[0], w2.shape[2]), dtype=np.float32)
    for e in range(w1.shape[0]):
        xe = x[offs[e]:offs[e + 1]]
        u = xe @ w1[e]
        g = xe @ w3[e]
        z = (u / (1.0 + np.exp(-u))) * g
        out[offs[e]:offs[e + 1]] = z @ w2[e]
    return out


def kernel(x, w1, w2, w3, m_sizes, _trace=False, _trace_kwargs=None):
    global LAST_RESULTS
    x = np.ascontiguousarray(x, dtype=np.float32)
    w1 = np.ascontiguousarray(w1, dtype=np.float32)
    w2 = np.ascontiguousarray(w2, dtype=np.float32)
    w3 = np.ascontiguousarray(w3, dtype=np.float32)
    m = np.asarray(m_sizes, dtype=np.int64)

    expected = (
        x.shape == (T, D)
        and w1.shape == (E, D, H)
        and w2.shape == (E, H, D)
        and w3.shape == (E, D, H)
        and m.shape == (E,)
        and np.all(m == M)
    )
    if not expected:
        return _numpy_fallback(x, w1, w2, w3, m_sizes)

    from concourse.bass_utils import run_bass_kernel_spmd

    bf = ml_dtypes.bfloat16
    nc = _get_program()
    in_maps = []
    for e in range(E):
        in_maps.append({
            "xT": np.ascontiguousarray(x[e * M:(e + 1) * M].T.astype(bf)),
            "w1r": _prep_w13(w1[e].astype(bf)),
            "w3r": _prep_w13(w3[e].astype(bf)),
            "w2r": np.ascontiguousarray(
                w2[e].astype(bf).reshape(HC, P, D)
            ),
        })

    res = run_bass_kernel_spmd(
        nc, in_maps, core_ids=list(range(E)),
        trace=_trace, **(_trace_kwargs or {}),
    )
    LAST_RESULTS = res
    return np.concatenate(
        [r["out"].astype(np.float32) for r in res.results], axis=0
    )
